# revision 1
# baseline (speedup 1.0000x reference)
"""Trainium2 Bass kernel for nn_FactorizedCrossAttention.

Key algebraic facts used (verified against the reference in fp64):
  * The "spatial" and "temporal" branches compute IDENTICAL per-position
    values: cross-attention over text tokens is independent per query row,
    and qt rows equal qs rows (same x row through the same Wq).  Hence
    spatial == temporal exactly.
  * concat([A, A]) @ Wst @ Wo == A @ ((Wst[:D] + Wst[D:]) @ Wo) — so both
    output projections fold into one 1024x1024 matrix Weff.
  * softmax scale (0.125) is folded into K on the host; the padding-mask
    bias is folded into an extra rank-1 accumulation matmul (skipped when
    the mask is all-True, which is the spec'd input).  No max-subtraction:
    scores are O(1) so exp cannot overflow.
  * softmax sums come from the PV matmul via a ones column appended to V
    (even heads) or a separate rank-1 ones matmul (odd heads), so no
    partition-dim reductions are needed.

Sharding: pure data-parallel over (B, T_frames): 32 frames / 8 cores =
4 frames (4096 query rows) per core; K/V/weights replicated.  No
collectives.

Device layout is "transposed activations": X^T, Q^T, A^T all live as
[feature-part, row-free] tiles so every matmul is a natural slice.  Head h
occupies partitions (h%2)*64..+64 of feature chunk h//2; K^T is replicated
on both partition halves so odd heads read lane-aligned operands, and odd
heads' PV output is placed at PSUM base 64 (tile_position) so the
normalized A^T lands on partitions 64..127 without any cross-partition
copies.
"""

import sys

if "/opt/trn_rl_repo" not in sys.path:
    sys.path.insert(0, "/opt/trn_rl_repo")

from contextlib import ExitStack

import ml_dtypes
import numpy as np

import concourse.bass as bass
import concourse.mybir as mybir
import concourse.tile as tile
from concourse import bacc
from concourse.bass_utils import run_bass_kernel_spmd

BF16 = ml_dtypes.bfloat16

D = 1024           # d_model
H = 16             # num heads
G = 4              # query groups
HD = 64            # head dim
HPG = H // G       # heads per group
SCALE = 0.125
B, T, HW, TT = 2, 16, 1024, 77
NCORES = 8
FPC = (B * T) // NCORES      # frames per core = 4
ROWS = FPC * HW              # 4096 query rows per core
RT = 512                     # rows per row-tile
NRT = ROWS // RT             # 8
NK = D // 128                # 8 partition chunks of d_model
VA = HD + 1                  # v columns + ones column

_PROG_CACHE = {}


def _patch_act_tables():
    """Force every activation onto the one table set that contains Exp, Ln
    and Copy together (natural_log_exp_and_others, same 400-interval
    precision).  Without this, bacc's table-load pass alternates between the
    exp-only and ln-only sets, costing a ~1.28us ACT_TABLE_LOAD per switch
    (~330us/core measured)."""
    import concourse.bacc as _bm
    import concourse.hw_specs as _hw
    if getattr(_bm, "_act_tables_patched", False):
        return
    _orig = _hw.get_activation_tables

    def patched(arch):
        t = dict(_orig(arch))
        combo = None
        for name, funcs in t.items():
            if (mybir.ActivationFunctionType.Exp in funcs
                    and mybir.ActivationFunctionType.Ln in funcs):
                combo = name
                break
        if combo is not None:
            for name in list(t):
                if name != combo:
                    t[name] = set()
        return t

    _bm.get_activation_tables = patched
    _bm._act_tables_patched = True

# test.py can flip these for profiling runs
TRACE = False
TRACE_KWARGS = {}
LAST_RESULTS = None


def _build_program(with_bias: bool):
    _patch_act_tables()
    dt = mybir.dt
    nc = bacc.Bacc("TRN2", target_bir_lowering=False, debug=False,
                   num_devices=NCORES)

    xt = nc.dram_tensor("xt", [D, ROWS], dt.bfloat16, kind="ExternalInput").ap()
    wq = nc.dram_tensor("wq", [D, D], dt.bfloat16, kind="ExternalInput").ap()
    weff = nc.dram_tensor("weff", [D, D], dt.bfloat16, kind="ExternalInput").ap()
    # K^T replicated on both partition halves: [128, G*TT]
    kt = nc.dram_tensor("kt", [128, G * TT], dt.bfloat16, kind="ExternalInput").ap()
    # V with a trailing ones column per group: [TT, G*(HD+1)]
    vaug = nc.dram_tensor("vaug", [TT, G * VA], dt.bfloat16, kind="ExternalInput").ap()
    if with_bias:
        biasr = nc.dram_tensor("biasr", [1, TT], dt.bfloat16, kind="ExternalInput").ap()
    out = nc.dram_tensor("out", [ROWS, D], dt.float32, kind="ExternalOutput").ap()

    with tile.TileContext(nc) as tc, ExitStack() as ctx:
        wpool = ctx.enter_context(tc.tile_pool(name="weights", bufs=1))
        xpool = ctx.enter_context(tc.tile_pool(name="xt", bufs=2))
        qpool = ctx.enter_context(tc.tile_pool(name="qt", bufs=2))
        apool = ctx.enter_context(tc.tile_pool(name="at", bufs=2))
        aupool = ctx.enter_context(tc.tile_pool(name="au", bufs=2))
        ppool = ctx.enter_context(tc.tile_pool(name="pt", bufs=3))
        supool = ctx.enter_context(tc.tile_pool(name="sumsb", bufs=6))
        lnpool = ctx.enter_context(tc.tile_pool(name="lnt", bufs=6))
        rpool = ctx.enter_context(tc.tile_pool(name="recip", bufs=6))
        opool = ctx.enter_context(tc.tile_pool(name="osb", bufs=3))
        # 8 PSUM banks: qp(2) + sprb(2, scores+bcast) + ap(2) + op(2)
        qpsum = ctx.enter_context(tc.tile_pool(name="qpsum", bufs=2, space="PSUM"))
        spsum = ctx.enter_context(tc.tile_pool(name="spsum", bufs=2, space="PSUM"))
        apsum = ctx.enter_context(tc.tile_pool(name="apsum", bufs=2, space="PSUM"))
        opsum = ctx.enter_context(tc.tile_pool(name="opsum", bufs=2, space="PSUM"))

        # --- resident weights ---
        wq_t = wpool.tile([128, NK * D], dt.bfloat16, tag="wq")
        weff_t = wpool.tile([128, NK * D], dt.bfloat16, tag="weff")
        for kc in range(NK):
            nc.sync.dma_start(out=wq_t[:, kc * D:(kc + 1) * D],
                              in_=wq[kc * 128:(kc + 1) * 128, :])
            nc.sync.dma_start(out=weff_t[:, kc * D:(kc + 1) * D],
                              in_=weff[kc * 128:(kc + 1) * 128, :])
        kt_t = wpool.tile([128, G * TT], dt.bfloat16, tag="kt")
        nc.sync.dma_start(out=kt_t[:], in_=kt[:, :])
        vaug_t = wpool.tile([TT, G * VA], dt.bfloat16, tag="vaug")
        nc.sync.dma_start(out=vaug_t[:], in_=vaug[:, :])
        ones77_t = wpool.tile([TT, 1], dt.bfloat16, tag="ones77")
        nc.vector.memset(ones77_t[:], 1.0)
        ones64_t = wpool.tile([128, HD], dt.bfloat16, tag="ones64")
        nc.vector.memset(ones64_t[:], 1.0)
        if with_bias:
            bias_t = wpool.tile([1, TT], dt.bfloat16, tag="bias")
            nc.sync.dma_start(out=bias_t[:], in_=biasr[:, :])
            ones_t = wpool.tile([1, RT], dt.bfloat16, tag="ones")
            nc.vector.memset(ones_t[:], 1.0)

        prev = None
        for rt in range(NRT):
            rsl = bass.ts(rt, RT)  # row slice in DRAM

            # --- load X^T row-tile: [1024 din, 512 rows] packed [128, 8*512]
            xt_t = xpool.tile([128, NK * RT], dt.bfloat16, tag="xt")
            for kc in range(NK):
                nc.sync.dma_start(
                    out=xt_t[:, kc * RT:(kc + 1) * RT],
                    in_=xt[kc * 128:(kc + 1) * 128, rsl],
                )

            # --- Q^T = Wq^T @ X^T : [1024 qcols, 512 rows] packed [128, 8*512]
            qt_t = qpool.tile([128, NK * RT], dt.bfloat16, tag="qt")
            for cc in range(NK):
                qp = qpsum.tile([128, RT], dt.float32, tag="qp")
                for kc in range(NK):
                    nc.tensor.matmul(
                        qp[:],
                        lhsT=wq_t[:, kc * D + cc * 128: kc * D + (cc + 1) * 128],
                        rhs=xt_t[:, kc * RT:(kc + 1) * RT],
                        start=(kc == 0), stop=(kc == NK - 1),
                    )
                nc.vector.tensor_copy(qt_t[:, cc * RT:(cc + 1) * RT], qp[:])

            # --- attention per head -> A^T packed [128, 8*512] (bf16)
            # sums ride the PV matmul (ones column for even heads at lane 64,
            # rank-1 ones matmul at lane 32 for odd heads); 1/s = exp(-ln s)
            # on ACT (all three ACT funcs live in one table set, see
            # _patch_act_tables), broadcast along the head's 64 partitions
            # with a rank-1 outer-product matmul, then DVE multiply.
            at_t = apool.tile([128, NK * RT], dt.bfloat16, tag="at")

            def wproj_chunk(prt, pat, rc4):
                # one 128-row output chunk of the PREVIOUS rowtile's Out
                ot = opool.tile([128, D], dt.float32, tag="ot")
                for oc in range(2):
                    op_ = opsum.tile([128, RT], dt.float32, tag="op")
                    for ac in range(NK):
                        nc.tensor.matmul(
                            op_[:],
                            lhsT=pat[:, ac * RT + rc4 * 128: ac * RT + (rc4 + 1) * 128],
                            rhs=weff_t[:, ac * D + oc * 512: ac * D + (oc + 1) * 512],
                            start=(ac == 0), stop=(ac == NK - 1),
                        )
                    nc.vector.tensor_copy(ot[:, oc * 512:(oc + 1) * 512], op_[:])
                nc.sync.dma_start(
                    out=out[prt * RT + rc4 * 128: prt * RT + (rc4 + 1) * 128, :],
                    in_=ot[:],
                )

            for h in range(H):
                g = h // HPG
                cc = h // 2
                po = (h % 2) * HD          # partition base of this head's Q/A
                csl = bass.ds(cc * RT, RT)  # column slice of the packed tiles

                sp = spsum.tile([128, RT], dt.float32, tag="sprb")
                nc.tensor.matmul(
                    sp[0:TT, :],
                    lhsT=kt_t[po:po + HD, g * TT:(g + 1) * TT],
                    rhs=qt_t[po:po + HD, csl],
                    start=True, stop=not with_bias,
                )
                if with_bias:
                    nc.tensor.matmul(
                        sp[0:TT, :], lhsT=bias_t[:, :], rhs=ones_t[:, :],
                        start=False, stop=True,
                    )
                pt = ppool.tile([TT, RT], dt.bfloat16, tag="pt")
                nc.scalar.activation(pt[:], sp[0:TT, :],
                                     mybir.ActivationFunctionType.Exp)

                ap_ = apsum.tile([128, RT], dt.float32, tag="ap")
                if po == 0:
                    nc.tensor.matmul(
                        ap_[0:VA, :],
                        lhsT=vaug_t[:, g * VA:(g + 1) * VA],
                        rhs=pt[:],
                        start=True, stop=True,
                    )
                    slane = HD
                else:
                    nc.tensor.matmul(
                        ap_[HD:2 * HD, :],
                        lhsT=vaug_t[:, g * VA:g * VA + HD],
                        rhs=pt[:],
                        start=True, stop=True,
                    )
                    nc.tensor.matmul(
                        ap_[32:33, :],
                        lhsT=ones77_t[:, :],
                        rhs=pt[:],
                        start=True, stop=True,
                    )
                    slane = 32
                lnt = lnpool.tile([128, RT], dt.float32, tag="lnt")
                nc.scalar.activation(lnt[slane:slane + 1, :],
                                     ap_[slane:slane + 1, :],
                                     mybir.ActivationFunctionType.Ln)
                rc = rpool.tile([128, RT], dt.bfloat16, tag="rc")
                nc.scalar.activation(rc[slane:slane + 1, :],
                                     lnt[slane:slane + 1, :],
                                     mybir.ActivationFunctionType.Exp,
                                     scale=-1.0)
                rb = spsum.tile([128, RT], dt.float32, tag="sprb")
                nc.tensor.matmul(
                    rb[po:po + HD, :],
                    lhsT=ones64_t[slane:slane + 1, :],
                    rhs=rc[slane:slane + 1, :],
                    start=True, stop=True,
                )
                rbs = rpool.tile([128, RT], dt.float32, tag="rbs")
                nc.vector.tensor_copy(rbs[po:po + HD, :], rb[po:po + HD, :])
                nc.vector.tensor_mul(
                    at_t[po:po + HD, csl],
                    ap_[po:po + HD, :], rbs[po:po + HD, :],
                )
                # software pipeline: previous rowtile's Wproj chunks are
                # interleaved between attention heads so the PE has dense
                # independent matmul work during the ACT/DVE softmax chains
                # (keeps the HAM clock gate warm).
                if prev is not None and h % 4 == 3:
                    wproj_chunk(prev[0], prev[1], h // 4)

            prev = (rt, at_t)

        # drain: Wproj of the final rowtile
        for rc4 in range(RT // 128):
            wproj_chunk(prev[0], prev[1], rc4)

    nc.compile()
    return nc


def _get_program(with_bias: bool):
    if with_bias not in _PROG_CACHE:
        _PROG_CACHE[with_bias] = _build_program(with_bias)
    return _PROG_CACHE[with_bias]


def _prep_inputs(x, te, mask, Wq, Wk, Wv, Wo, Wst):
    """Host-side fp32 weight prep + per-core shard maps."""
    K = (te @ Wk).reshape(B, TT, G, HD) * SCALE
    V = (te @ Wv).reshape(B, TT, G, HD)
    Weff = ((Wst[:D] + Wst[D:]) @ Wo).astype(np.float32)

    wq_b = Wq.astype(BF16)
    weff_b = Weff.astype(BF16)
    kt_b, vaug_b, bias_b = [], [], []
    for b in range(B):
        k64 = np.ascontiguousarray(
            K[b].transpose(2, 1, 0)).reshape(HD, G * TT).astype(BF16)
        kt_b.append(np.concatenate([k64, k64], axis=0))
        va = np.concatenate([V[b], np.ones((TT, G, 1), np.float32)], axis=2)
        vaug_b.append(np.ascontiguousarray(va).reshape(TT, G * VA).astype(BF16))
        bias_b.append(np.where(mask[b], 0.0, -30.0).astype(BF16).reshape(1, TT))

    with_bias = not bool(mask.all())
    in_maps = []
    for c in range(NCORES):
        b = c // (NCORES // B)
        fr = (c % (NCORES // B)) * FPC
        xc = x[b, fr:fr + FPC].reshape(ROWS, D).astype(BF16)
        m = {
            "xt": np.ascontiguousarray(xc.T),
            "wq": wq_b,
            "weff": weff_b,
            "kt": kt_b[b],
            "vaug": vaug_b[b],
        }
        if with_bias:
            m["biasr"] = bias_b[b]
        in_maps.append(m)
    return with_bias, in_maps


def kernel(x, text_embeddings, padding_mask, use_mqa=0, use_qk_norm=0,
           Wq=None, Wk=None, Wv=None, Wo=None, Wst=None):
    global LAST_RESULTS
    x = np.asarray(x, np.float32)
    te = np.asarray(text_embeddings, np.float32)
    mask = np.asarray(padding_mask).astype(bool)
    Wq = np.asarray(Wq, np.float32)
    Wk = np.asarray(Wk, np.float32)
    Wv = np.asarray(Wv, np.float32)
    Wo = np.asarray(Wo, np.float32)
    Wst = np.asarray(Wst, np.float32)
    assert x.shape == (B, T, HW, D) and te.shape == (B, TT, D)

    with_bias, in_maps = _prep_inputs(x, te, mask, Wq, Wk, Wv, Wo, Wst)
    nc = _get_program(with_bias)

    res = run_bass_kernel_spmd(nc, in_maps, list(range(NCORES)),
                               trace=TRACE, **TRACE_KWARGS)
    LAST_RESULTS = res

    outp = np.empty((B, T, HW, D), np.float32)
    for c in range(NCORES):
        b = c // (NCORES // B)
        fr = (c % (NCORES // B)) * FPC
        outp[b, fr:fr + FPC] = res.results[c]["out"].reshape(FPC, HW, D)
    return outp



# revision 3
# speedup vs baseline: 1.9074x; 1.9074x over previous
"""Trainium2 Bass kernel for nn_FactorizedCrossAttention.

Algebraic restructure (verified exact in fp32 vs the reference):
  * spatial == temporal (cross-attention is per-row; qt rows == qs rows), so
    concat([A,A]) @ Wst @ Wo == A @ Weff with Weff = (Wst[:D]+Wst[D:]) @ Wo.
  * Q is never materialized: scores = X @ M with M_h = Wq_h @ (K_h*scale)^T
    folded on the host ([1024, 16*77] packed at stride 80 -> [1024, 1280]).
  * Weff is folded into V on the host: Vt_h = V_h @ Weff[64h:64h+64, :], so
    out = P_norm @ Vt sums over all (head, token) pairs in one GEMM.
  * softmax: S^T tiles keep tokens on partitions, so the padding mask is a
    free per-partition bias on the ACT exp.  Row sums for ALL heads land in
    one [16, 512] PSUM tile via block-indicator matmuls; one DVE reciprocal
    + selector matmuls broadcast 1/s back to token partitions.

Device layout: everything "transposed" ([feature/token part, row free]) until
the final GEMM, which uses P_norm^T as the stationary operand so the output
comes out row-major for dense DMA.

Scheduling: per 512-row tile the PE runs 180 uniform 512-cycle matmuls
(80 scores + 10 sums + 10 bcast + 80 PV*Weff of the previous row tile)
with the previous row tile's GEMM interleaved to cover the softmax
reciprocal latency, keeping the PE out of its low p-states.

Sharding: pure data-parallel over (B, T_frames): 32 frames / 8 cores.
No collectives.
"""

import sys

if "/opt/trn_rl_repo" not in sys.path:
    sys.path.insert(0, "/opt/trn_rl_repo")

from contextlib import ExitStack

import ml_dtypes
import numpy as np

import concourse.bass as bass
import concourse.mybir as mybir
import concourse.tile as tile
from concourse import bacc
from concourse.bass_utils import run_bass_kernel_spmd

BF16 = ml_dtypes.bfloat16

D = 1024           # d_model
H = 16             # num heads
G = 4              # query groups
HD = 64            # head dim
HPG = H // G
SCALE = 0.125
B, T, HW, TT = 2, 16, 1024, 77
NCORES = 8
FPC = (B * T) // NCORES      # frames per core = 4
ROWS = FPC * HW              # 4096 query rows per core
RT = 512                     # rows per row-tile
NRT = ROWS // RT             # 8
ND = D // 128                # 8 partition chunks of d_model
HS = 80                      # per-head stride in the packed token axis
NT = H * HS                  # 1280 packed (head, token) rows
NTT = NT // 128              # 10 token tiles

_PROG_CACHE = {}


def _patch_act_tables():
    """Pin every activation to the one table set containing Exp and Copy so
    bacc never emits mid-kernel ACT_TABLE_LOAD switches."""
    import concourse.bacc as _bm
    import concourse.hw_specs as _hw
    if getattr(_bm, "_act_tables_patched", False):
        return
    _orig = _hw.get_activation_tables

    def patched(arch):
        t = dict(_orig(arch))
        combo = None
        for name, funcs in t.items():
            if (mybir.ActivationFunctionType.Exp in funcs
                    and mybir.ActivationFunctionType.Copy in funcs):
                combo = name
                break
        if combo is not None:
            for name in list(t):
                if name != combo:
                    t[name] = set()
        return t

    _bm.get_activation_tables = patched
    _bm._act_tables_patched = True

# test.py can flip these for profiling runs
TRACE = False
TRACE_KWARGS = {}
LAST_RESULTS = None


def _build_program():
    _patch_act_tables()
    dt = mybir.dt
    nc = bacc.Bacc("TRN2", target_bir_lowering=False, debug=False,
                   num_devices=NCORES)

    xt = nc.dram_tensor("xt", [D, ROWS], dt.bfloat16, kind="ExternalInput").ap()
    mt = nc.dram_tensor("mt", [D, NT], dt.bfloat16, kind="ExternalInput").ap()
    vt = nc.dram_tensor("vt", [NT, D], dt.bfloat16, kind="ExternalInput").ap()
    ind = nc.dram_tensor("ind", [128, NTT * H], dt.bfloat16, kind="ExternalInput").ap()
    sel = nc.dram_tensor("sel", [H, NTT * 128], dt.bfloat16, kind="ExternalInput").ap()
    ebias = nc.dram_tensor("ebias", [128, NTT], dt.float32, kind="ExternalInput").ap()
    out = nc.dram_tensor("out", [ROWS, D], dt.float32, kind="ExternalOutput").ap()

    with tile.TileContext(nc) as tc, ExitStack() as ctx:
        wpool = ctx.enter_context(tc.tile_pool(name="weights", bufs=1))
        xpool = ctx.enter_context(tc.tile_pool(name="xt", bufs=2))
        ptpool = ctx.enter_context(tc.tile_pool(name="pt", bufs=2))
        pnpool = ctx.enter_context(tc.tile_pool(name="pn", bufs=2))
        rcpool = ctx.enter_context(tc.tile_pool(name="recip", bufs=2))
        otpool = ctx.enter_context(tc.tile_pool(name="osb", bufs=3))
        # 8 PSUM banks: scores(2) + sums(2) + bcast(2) + pvw out(2)
        spsum = ctx.enter_context(tc.tile_pool(name="spsum", bufs=2, space="PSUM"))
        supsum = ctx.enter_context(tc.tile_pool(name="supsum", bufs=2, space="PSUM"))
        rbpsum = ctx.enter_context(tc.tile_pool(name="rbpsum", bufs=2, space="PSUM"))
        opsum = ctx.enter_context(tc.tile_pool(name="opsum", bufs=2, space="PSUM"))

        # --- resident weights ---
        mt_t = wpool.tile([128, ND * NT], dt.bfloat16, tag="mt")
        for kc in range(ND):
            nc.sync.dma_start(out=mt_t[:, kc * NT:(kc + 1) * NT],
                              in_=mt[kc * 128:(kc + 1) * 128, :])
        vt_t = wpool.tile([128, NTT * D], dt.bfloat16, tag="vt")
        for st in range(NTT):
            nc.sync.dma_start(out=vt_t[:, st * D:(st + 1) * D],
                              in_=vt[st * 128:(st + 1) * 128, :])
        ind_t = wpool.tile([128, NTT * H], dt.bfloat16, tag="ind")
        nc.sync.dma_start(out=ind_t[:], in_=ind[:, :])
        sel_t = wpool.tile([H, NTT * 128], dt.bfloat16, tag="sel")
        nc.sync.dma_start(out=sel_t[:], in_=sel[:, :])
        ebias_t = wpool.tile([128, NTT], dt.float32, tag="ebias")
        nc.sync.dma_start(out=ebias_t[:], in_=ebias[:, :])

        def pvw_quarter(pat, prt, rc):
            """One 128-row chunk of the previous rowtile's P_norm @ Vt."""
            ot = otpool.tile([128, D], dt.float32, tag="ot")
            for oc in range(2):
                op_ = opsum.tile([128, RT], dt.float32, tag="op")
                for st in range(NTT):
                    nc.tensor.matmul(
                        op_[:],
                        lhsT=pat[:, st * RT + rc * 128: st * RT + (rc + 1) * 128],
                        rhs=vt_t[:, st * D + oc * RT: st * D + (oc + 1) * RT],
                        start=(st == 0), stop=(st == NTT - 1),
                    )
                if oc == 0:
                    nc.scalar.copy(ot[:, oc * RT:(oc + 1) * RT], op_[:])
                else:
                    nc.vector.tensor_copy(ot[:, oc * RT:(oc + 1) * RT], op_[:])
            nc.sync.dma_start(
                out=out[prt * RT + rc * 128: prt * RT + (rc + 1) * 128, :],
                in_=ot[:],
            )

        prev = None
        for rt in range(NRT):
            # --- load X^T row-tile as [128, 8, 512] in one DMA
            xt_t = xpool.tile([128, ND * RT], dt.bfloat16, tag="xt")
            nc.sync.dma_start(
                out=xt_t[:].rearrange("p (k r) -> p k r", k=ND),
                in_=xt[:, rt * RT:(rt + 1) * RT].rearrange(
                    "(k p) r -> p k r", p=128),
            )

            # --- scores + exp per token tile
            pt_t = ptpool.tile([128, NTT * RT], dt.bfloat16, tag="pt")
            for st in range(NTT):
                sp = spsum.tile([128, RT], dt.float32, tag="sp")
                for kc in range(ND):
                    nc.tensor.matmul(
                        sp[:],
                        lhsT=mt_t[:, kc * NT + st * 128: kc * NT + (st + 1) * 128],
                        rhs=xt_t[:, kc * RT:(kc + 1) * RT],
                        start=(kc == 0), stop=(kc == ND - 1),
                    )
                nc.scalar.activation(pt_t[:, st * RT:(st + 1) * RT], sp[:],
                                     mybir.ActivationFunctionType.Exp,
                                     bias=ebias_t[:, st:st + 1])

            # --- all-head row sums -> [16, 512] PSUM
            su = supsum.tile([H, RT], dt.float32, tag="su")
            for st in range(NTT):
                nc.tensor.matmul(
                    su[:],
                    lhsT=ind_t[:, st * H:(st + 1) * H],
                    rhs=pt_t[:, st * RT:(st + 1) * RT],
                    start=(st == 0), stop=(st == NTT - 1),
                )
            rc_f = rcpool.tile([H, RT], dt.float32, tag="rcf")
            nc.vector.reciprocal(rc_f[:], su[:])
            rc_b = rcpool.tile([H, RT], dt.bfloat16, tag="rcb")
            nc.vector.tensor_copy(rc_b[:], rc_f[:])

            # cover the reciprocal latency with independent PVW work
            if prev is not None:
                pvw_quarter(prev[1], prev[0], 0)

            # --- broadcast 1/s to token partitions, normalize P
            pn_t = pnpool.tile([128, NTT * RT], dt.bfloat16, tag="pn")
            for st in range(NTT):
                rb = rbpsum.tile([128, RT], dt.float32, tag="rb")
                nc.tensor.matmul(
                    rb[:],
                    lhsT=sel_t[:, st * 128:(st + 1) * 128],
                    rhs=rc_b[:],
                    start=True, stop=True,
                )
                nc.vector.tensor_mul(
                    pn_t[:, st * RT:(st + 1) * RT],
                    pt_t[:, st * RT:(st + 1) * RT], rb[:],
                )

            if prev is not None:
                for rc4 in range(1, 4):
                    pvw_quarter(prev[1], prev[0], rc4)
            prev = (rt, pn_t)

        # drain: PVW of the final rowtile
        for rc4 in range(4):
            pvw_quarter(prev[1], prev[0], rc4)

    nc.compile()
    return nc


def _get_program():
    if "p" not in _PROG_CACHE:
        _PROG_CACHE["p"] = _build_program()
    return _PROG_CACHE["p"]


def _prep_inputs(x, te, mask, Wq, Wk, Wv, Wo, Wst):
    """Host-side fp32 weight folding + per-core shard maps."""
    Weff = ((Wst[:D] + Wst[D:]) @ Wo).astype(np.float32)

    # packed-token-axis structure: global row g -> head g//HS, token g%HS
    g = np.arange(NT)
    head_of = g // HS
    tok_of = g % HS
    real = tok_of < TT

    ind_np = np.zeros((128, NTT * H), np.float32)
    sel_np = np.zeros((H, NTT * 128), np.float32)
    for st in range(NTT):
        for p in range(128):
            gg = st * 128 + p
            if real[gg]:
                ind_np[p, st * H + head_of[gg]] = 1.0
                sel_np[head_of[gg], st * 128 + p] = 1.0

    mt_b, vt_b, eb_b = [], [], []
    for b in range(B):
        K = ((te[b] @ Wk).reshape(TT, G, HD) * SCALE).astype(np.float32)
        V = (te[b] @ Wv).reshape(TT, G, HD).astype(np.float32)
        Mp = np.zeros((D, NT), np.float32)
        Vp = np.zeros((NT, D), np.float32)
        for h in range(H):
            gq = h // HPG
            Mp[:, h * HS:h * HS + TT] = Wq[:, h * HD:(h + 1) * HD] @ K[:, gq, :].T
            Vp[h * HS:h * HS + TT] = V[:, gq, :] @ Weff[h * HD:(h + 1) * HD, :]
        mt_b.append(Mp.astype(BF16))
        vt_b.append(Vp.astype(BF16))
        # exp bias: 0 for attended tokens, -30 for masked/pad rows
        eb = np.full(NT, -30.0, np.float32)
        eb[real] = np.where(mask[b][tok_of[real]], 0.0, -30.0)
        eb_b.append(np.ascontiguousarray(eb.reshape(NTT, 128).T))

    ind_np = ind_np.astype(BF16)
    sel_np = sel_np.astype(BF16)

    in_maps = []
    for c in range(NCORES):
        b = c // (NCORES // B)
        fr = (c % (NCORES // B)) * FPC
        xc = x[b, fr:fr + FPC].reshape(ROWS, D).astype(BF16)
        in_maps.append({
            "xt": np.ascontiguousarray(xc.T),
            "mt": mt_b[b],
            "vt": vt_b[b],
            "ind": ind_np,
            "sel": sel_np,
            "ebias": eb_b[b],
        })
    return in_maps


def kernel(x, text_embeddings, padding_mask, use_mqa=0, use_qk_norm=0,
           Wq=None, Wk=None, Wv=None, Wo=None, Wst=None):
    global LAST_RESULTS
    x = np.asarray(x, np.float32)
    te = np.asarray(text_embeddings, np.float32)
    mask = np.asarray(padding_mask).astype(bool)
    Wq = np.asarray(Wq, np.float32)
    Wk = np.asarray(Wk, np.float32)
    Wv = np.asarray(Wv, np.float32)
    Wo = np.asarray(Wo, np.float32)
    Wst = np.asarray(Wst, np.float32)
    assert x.shape == (B, T, HW, D) and te.shape == (B, TT, D)

    in_maps = _prep_inputs(x, te, mask, Wq, Wk, Wv, Wo, Wst)
    nc = _get_program()

    res = run_bass_kernel_spmd(nc, in_maps, list(range(NCORES)),
                               trace=TRACE, **TRACE_KWARGS)
    LAST_RESULTS = res

    outp = np.empty((B, T, HW, D), np.float32)
    for c in range(NCORES):
        b = c // (NCORES // B)
        fr = (c % (NCORES // B)) * FPC
        outp[b, fr:fr + FPC] = res.results[c]["out"].reshape(FPC, HW, D)
    return outp


# revision 8
# speedup vs baseline: 1.9289x; 1.0113x over previous
"""Trainium2 Bass kernel for nn_FactorizedCrossAttention.

Algebraic restructure (verified exact in fp32 vs the reference):
  * spatial == temporal (cross-attention is per-row; qt rows == qs rows), so
    concat([A,A]) @ Wst @ Wo == A @ Weff with Weff = (Wst[:D]+Wst[D:]) @ Wo.
  * Q is never materialized: scores = X @ M with M_h = Wq_h @ (K_h*scale)^T
    folded on the host ([1024, 16*77] packed at stride 80 -> [1024, 1280]).
  * Weff is folded into V on the host: Vt_h = V_h @ Weff[64h:64h+64, :], so
    out = P_norm @ Vt sums over all (head, token) pairs in one GEMM.
  * softmax: S^T tiles keep tokens on partitions, so the padding mask is a
    free per-partition bias on the ACT exp.  Row sums for ALL heads land in
    one [16, 512] PSUM tile via block-indicator matmuls; one DVE reciprocal
    + selector matmuls broadcast 1/s back to token partitions.

Device layout: everything "transposed" ([feature/token part, row free]) until
the final GEMM, which uses P_norm^T as the stationary operand so the output
comes out row-major for dense DMA.

Scheduling: per 512-row tile the PE runs 180 uniform 512-cycle matmuls
(80 scores + 10 sums + 10 bcast + 80 PV*Weff of the previous row tile)
with the previous row tile's GEMM interleaved to cover the softmax
reciprocal latency, keeping the PE out of its low p-states.

Sharding: pure data-parallel over (B, T_frames): 32 frames / 8 cores.
No collectives.
"""

import sys

if "/opt/trn_rl_repo" not in sys.path:
    sys.path.insert(0, "/opt/trn_rl_repo")

from contextlib import ExitStack

import ml_dtypes
import numpy as np

import concourse.bass as bass
import concourse.mybir as mybir
import concourse.tile as tile
from concourse import bacc
from concourse.bass_utils import run_bass_kernel_spmd

BF16 = ml_dtypes.bfloat16

D = 1024           # d_model
H = 16             # num heads
G = 4              # query groups
HD = 64            # head dim
HPG = H // G
SCALE = 0.125
B, T, HW, TT = 2, 16, 1024, 77
NCORES = 8
FPC = (B * T) // NCORES      # frames per core = 4
ROWS = FPC * HW              # 4096 query rows per core
RT = 512                     # rows per row-tile
NRT = ROWS // RT             # 8
ND = D // 128                # 8 partition chunks of d_model
HS = 80                      # per-head stride in the packed token axis
NT = H * HS                  # 1280 packed (head, token) rows
NTT = NT // 128              # 10 token tiles

_PROG_CACHE = {}


def _patch_act_tables():
    """Pin every activation to the one table set containing Exp and Copy so
    bacc never emits mid-kernel ACT_TABLE_LOAD switches."""
    import concourse.bacc as _bm
    import concourse.hw_specs as _hw
    if getattr(_bm, "_act_tables_patched", False):
        return
    _orig = _hw.get_activation_tables

    def patched(arch):
        t = dict(_orig(arch))
        combo = None
        for name, funcs in t.items():
            if (mybir.ActivationFunctionType.Exp in funcs
                    and mybir.ActivationFunctionType.Ln in funcs
                    and mybir.ActivationFunctionType.Copy in funcs):
                combo = name
                break
        if combo is not None:
            for name in list(t):
                if name != combo:
                    t[name] = set()
        return t

    _bm.get_activation_tables = patched
    _bm._act_tables_patched = True

# test.py can flip these for profiling runs
TRACE = False
TRACE_KWARGS = {}
LAST_RESULTS = None


def _build_program():
    _patch_act_tables()
    dt = mybir.dt
    nc = bacc.Bacc("TRN2", target_bir_lowering=False, debug=False,
                   num_devices=NCORES)

    xt = nc.dram_tensor("xt", [D, ROWS], dt.bfloat16, kind="ExternalInput").ap()
    mt = nc.dram_tensor("mt", [D, NT], dt.bfloat16, kind="ExternalInput").ap()
    vt = nc.dram_tensor("vt", [NT, D], dt.bfloat16, kind="ExternalInput").ap()
    ind = nc.dram_tensor("ind", [128, NTT * H], dt.bfloat16, kind="ExternalInput").ap()
    sel = nc.dram_tensor("sel", [H, NTT * 128], dt.bfloat16, kind="ExternalInput").ap()
    ebias = nc.dram_tensor("ebias", [128, NTT], dt.float32, kind="ExternalInput").ap()
    out = nc.dram_tensor("out", [ROWS, D], dt.float32, kind="ExternalOutput").ap()

    with tile.TileContext(nc) as tc, ExitStack() as ctx:
        wpool = ctx.enter_context(tc.tile_pool(name="weights", bufs=1))
        xpool = ctx.enter_context(tc.tile_pool(name="xt", bufs=2))
        ptpool = ctx.enter_context(tc.tile_pool(name="pt", bufs=2))
        pnpool = ctx.enter_context(tc.tile_pool(name="pn", bufs=2))
        rcpool = ctx.enter_context(tc.tile_pool(name="recip", bufs=2))
        otpool = ctx.enter_context(tc.tile_pool(name="osb", bufs=3))
        # 8 PSUM banks: scores(3) + sums(1) + bcast(2) + pvw out(2)
        spsum = ctx.enter_context(tc.tile_pool(name="spsum", bufs=3, space="PSUM"))
        supsum = ctx.enter_context(tc.tile_pool(name="supsum", bufs=1, space="PSUM"))
        rbpsum = ctx.enter_context(tc.tile_pool(name="rbpsum", bufs=2, space="PSUM"))
        opsum = ctx.enter_context(tc.tile_pool(name="opsum", bufs=2, space="PSUM"))

        # --- resident weights, loaded on the ACT hwdge queue so they overlap
        # the first xt load (sync queue).  mt is split per k-chunk so the
        # first score matmuls only wait for chunk 0.
        ebias_t = wpool.tile([128, NTT], dt.float32, tag="ebias")
        nc.scalar.dma_start(out=ebias_t[:], in_=ebias[:, :])
        mt_ts = []
        for kc in range(ND):
            mtc = wpool.tile([128, NT], dt.bfloat16, tag=f"mt{kc}")
            nc.scalar.dma_start(out=mtc[:], in_=mt[kc * 128:(kc + 1) * 128, :])
            mt_ts.append(mtc)
        ind_t = wpool.tile([128, NTT * H], dt.bfloat16, tag="ind")
        nc.scalar.dma_start(out=ind_t[:], in_=ind[:, :])
        sel_t = wpool.tile([H, NTT * 128], dt.bfloat16, tag="sel")
        nc.scalar.dma_start(out=sel_t[:], in_=sel[:, :])
        vt_t = wpool.tile([128, NTT * D], dt.bfloat16, tag="vt")
        for st in range(NTT):
            nc.scalar.dma_start(out=vt_t[:, st * D:(st + 1) * D],
                                in_=vt[st * 128:(st + 1) * 128, :])

        def pvw_quarter(pat, prt, rc):
            """One 128-row chunk of the previous rowtile's P_norm @ Vt."""
            ot = otpool.tile([128, D], dt.float32, tag="ot")
            for oc in range(2):
                op_ = opsum.tile([128, RT], dt.float32, tag="op")
                for st in range(NTT):
                    nc.tensor.matmul(
                        op_[:],
                        lhsT=pat[:, st * RT + rc * 128: st * RT + (rc + 1) * 128],
                        rhs=vt_t[:, st * D + oc * RT: st * D + (oc + 1) * RT],
                        start=(st == 0), stop=(st == NTT - 1),
                    )
                if oc == 0:
                    nc.scalar.copy(ot[:, oc * RT:(oc + 1) * RT], op_[:])
                else:
                    nc.vector.tensor_copy(ot[:, oc * RT:(oc + 1) * RT], op_[:])
            nc.sync.dma_start(
                out=out[prt * RT + rc * 128: prt * RT + (rc + 1) * 128, :],
                in_=ot[:],
            )

        prev = None
        for rt in range(NRT):
            # --- load X^T row-tile as [128, 8, 512] in one DMA
            xt_t = xpool.tile([128, ND * RT], dt.bfloat16, tag="xt")
            nc.sync.dma_start(
                out=xt_t[:].rearrange("p (k r) -> p k r", k=ND),
                in_=xt[:, rt * RT:(rt + 1) * RT].rearrange(
                    "(k p) r -> p k r", p=128),
            )

            # --- scores + exp per token tile
            pt_t = ptpool.tile([128, NTT * RT], dt.bfloat16, tag="pt")
            for st in range(NTT):
                sp = spsum.tile([128, RT], dt.float32, tag="sp")
                for kc in range(ND):
                    nc.tensor.matmul(
                        sp[:],
                        lhsT=mt_ts[kc][:, st * 128:(st + 1) * 128],
                        rhs=xt_t[:, kc * RT:(kc + 1) * RT],
                        start=(kc == 0), stop=(kc == ND - 1),
                    )
                nc.scalar.activation(pt_t[:, st * RT:(st + 1) * RT], sp[:],
                                     mybir.ActivationFunctionType.Exp,
                                     bias=ebias_t[:, st:st + 1])

            # --- all-head row sums -> [16, 512] PSUM
            su = supsum.tile([H, RT], dt.float32, tag="su")
            for st in range(NTT):
                nc.tensor.matmul(
                    su[:],
                    lhsT=ind_t[:, st * H:(st + 1) * H],
                    rhs=pt_t[:, st * RT:(st + 1) * RT],
                    start=(st == 0), stop=(st == NTT - 1),
                )
            # 1/s = exp(-ln s) on ACT (writes bf16 directly); the DVE
            # InstReciprocal measures ~3.3us and sits on the critical path.
            rc_f = rcpool.tile([H, RT], dt.float32, tag="rcf")
            nc.scalar.activation(rc_f[:], su[:],
                                 mybir.ActivationFunctionType.Ln)
            rc_b = rcpool.tile([H, RT], dt.bfloat16, tag="rcb")
            nc.scalar.activation(rc_b[:], rc_f[:],
                                 mybir.ActivationFunctionType.Exp,
                                 scale=-1.0)

            # cover the reciprocal latency with independent PVW work
            if prev is not None:
                pvw_quarter(prev[1], prev[0], 0)
                pvw_quarter(prev[1], prev[0], 1)

            # --- broadcast 1/s to token partitions, normalize P
            pn_t = pnpool.tile([128, NTT * RT], dt.bfloat16, tag="pn")
            for st in range(NTT):
                rb = rbpsum.tile([128, RT], dt.float32, tag="rb")
                nc.tensor.matmul(
                    rb[:],
                    lhsT=sel_t[:, st * 128:(st + 1) * 128],
                    rhs=rc_b[:],
                    start=True, stop=True,
                )
                nc.vector.tensor_mul(
                    pn_t[:, st * RT:(st + 1) * RT],
                    pt_t[:, st * RT:(st + 1) * RT], rb[:],
                )

            if prev is not None:
                for rc4 in range(2, 4):
                    pvw_quarter(prev[1], prev[0], rc4)
            prev = (rt, pn_t)

        # drain: PVW of the final rowtile
        for rc4 in range(4):
            pvw_quarter(prev[1], prev[0], rc4)

    nc.compile()
    return nc


def _get_program():
    if "p" not in _PROG_CACHE:
        _PROG_CACHE["p"] = _build_program()
    return _PROG_CACHE["p"]


def _prep_inputs(x, te, mask, Wq, Wk, Wv, Wo, Wst):
    """Host-side fp32 weight folding + per-core shard maps."""
    Weff = ((Wst[:D] + Wst[D:]) @ Wo).astype(np.float32)

    # packed-token-axis structure: global row g -> head g//HS, token g%HS
    g = np.arange(NT)
    head_of = g // HS
    tok_of = g % HS
    real = tok_of < TT

    ind_np = np.zeros((128, NTT * H), np.float32)
    sel_np = np.zeros((H, NTT * 128), np.float32)
    for st in range(NTT):
        for p in range(128):
            gg = st * 128 + p
            if real[gg]:
                ind_np[p, st * H + head_of[gg]] = 1.0
                sel_np[head_of[gg], st * 128 + p] = 1.0

    mt_b, vt_b, eb_b = [], [], []
    for b in range(B):
        K = ((te[b] @ Wk).reshape(TT, G, HD) * SCALE).astype(np.float32)
        V = (te[b] @ Wv).reshape(TT, G, HD).astype(np.float32)
        Mp = np.zeros((D, NT), np.float32)
        Vp = np.zeros((NT, D), np.float32)
        for h in range(H):
            gq = h // HPG
            Mp[:, h * HS:h * HS + TT] = Wq[:, h * HD:(h + 1) * HD] @ K[:, gq, :].T
            Vp[h * HS:h * HS + TT] = V[:, gq, :] @ Weff[h * HD:(h + 1) * HD, :]
        mt_b.append(Mp.astype(BF16))
        vt_b.append(Vp.astype(BF16))
        # exp bias: 0 for attended tokens, -30 for masked/pad rows
        eb = np.full(NT, -30.0, np.float32)
        eb[real] = np.where(mask[b][tok_of[real]], 0.0, -30.0)
        eb_b.append(np.ascontiguousarray(eb.reshape(NTT, 128).T))

    ind_np = ind_np.astype(BF16)
    sel_np = sel_np.astype(BF16)

    in_maps = []
    for c in range(NCORES):
        b = c // (NCORES // B)
        fr = (c % (NCORES // B)) * FPC
        xc = x[b, fr:fr + FPC].reshape(ROWS, D).astype(BF16)
        in_maps.append({
            "xt": np.ascontiguousarray(xc.T),
            "mt": mt_b[b],
            "vt": vt_b[b],
            "ind": ind_np,
            "sel": sel_np,
            "ebias": eb_b[b],
        })
    return in_maps


def kernel(x, text_embeddings, padding_mask, use_mqa=0, use_qk_norm=0,
           Wq=None, Wk=None, Wv=None, Wo=None, Wst=None):
    global LAST_RESULTS
    x = np.asarray(x, np.float32)
    te = np.asarray(text_embeddings, np.float32)
    mask = np.asarray(padding_mask).astype(bool)
    Wq = np.asarray(Wq, np.float32)
    Wk = np.asarray(Wk, np.float32)
    Wv = np.asarray(Wv, np.float32)
    Wo = np.asarray(Wo, np.float32)
    Wst = np.asarray(Wst, np.float32)
    assert x.shape == (B, T, HW, D) and te.shape == (B, TT, D)

    in_maps = _prep_inputs(x, te, mask, Wq, Wk, Wv, Wo, Wst)
    nc = _get_program()

    res = run_bass_kernel_spmd(nc, in_maps, list(range(NCORES)),
                               trace=TRACE, **TRACE_KWARGS)
    LAST_RESULTS = res

    outp = np.empty((B, T, HW, D), np.float32)
    for c in range(NCORES):
        b = c // (NCORES // B)
        fr = (c % (NCORES // B)) * FPC
        outp[b, fr:fr + FPC] = res.results[c]["out"].reshape(FPC, HW, D)
    return outp


# revision 11
# speedup vs baseline: 1.9388x; 1.0052x over previous
"""Trainium2 Bass kernel for nn_FactorizedCrossAttention.

Algebraic restructure (verified exact in fp32 vs the reference):
  * spatial == temporal (cross-attention is per-row; qt rows == qs rows), so
    concat([A,A]) @ Wst @ Wo == A @ Weff with Weff = (Wst[:D]+Wst[D:]) @ Wo.
  * Q is never materialized: scores = X @ M with M_h = Wq_h @ (K_h*scale)^T
    folded on the host ([1024, 16*77] packed at stride 80 -> [1024, 1280]).
  * Weff is folded into V on the host: Vt_h = V_h @ Weff[64h:64h+64, :], so
    out = P_norm @ Vt sums over all (head, token) pairs in one GEMM.
  * softmax: S^T tiles keep tokens on partitions, so the padding mask is a
    free per-partition bias on the ACT exp.  Row sums for ALL heads land in
    one [16, 512] PSUM tile via block-indicator matmuls; one DVE reciprocal
    + selector matmuls broadcast 1/s back to token partitions.

Device layout: everything "transposed" ([feature/token part, row free]) until
the final GEMM, which uses P_norm^T as the stationary operand so the output
comes out row-major for dense DMA.

Scheduling: per 512-row tile the PE runs 180 uniform 512-cycle matmuls
(80 scores + 10 sums + 10 bcast + 80 PV*Weff of the previous row tile)
with the previous row tile's GEMM interleaved to cover the softmax
reciprocal latency, keeping the PE out of its low p-states.

Sharding: pure data-parallel over (B, T_frames): 32 frames / 8 cores.
No collectives.
"""

import sys

if "/opt/trn_rl_repo" not in sys.path:
    sys.path.insert(0, "/opt/trn_rl_repo")

from contextlib import ExitStack

import ml_dtypes
import numpy as np

import concourse.bass as bass
import concourse.mybir as mybir
import concourse.tile as tile
from concourse import bacc
from concourse.bass_utils import run_bass_kernel_spmd

BF16 = ml_dtypes.bfloat16

D = 1024           # d_model
H = 16             # num heads
G = 4              # query groups
HD = 64            # head dim
HPG = H // G
SCALE = 0.125
B, T, HW, TT = 2, 16, 1024, 77
NCORES = 8
FPC = (B * T) // NCORES      # frames per core = 4
ROWS = FPC * HW              # 4096 query rows per core
RT = 512                     # rows per row-tile
NRT = ROWS // RT             # 8
ND = D // 128                # 8 partition chunks of d_model
HS = 80                      # per-head stride in the packed token axis
NT = H * HS                  # 1280 packed (head, token) rows
NTT = NT // 128              # 10 token tiles

_PROG_CACHE = {}


def _patch_act_tables():
    """Pin every activation to the one table set containing Exp and Copy so
    bacc never emits mid-kernel ACT_TABLE_LOAD switches."""
    import concourse.bacc as _bm
    import concourse.hw_specs as _hw
    if getattr(_bm, "_act_tables_patched", False):
        return
    _orig = _hw.get_activation_tables

    def patched(arch):
        t = dict(_orig(arch))
        combo = None
        for name, funcs in t.items():
            if (mybir.ActivationFunctionType.Exp in funcs
                    and mybir.ActivationFunctionType.Ln in funcs
                    and mybir.ActivationFunctionType.Copy in funcs):
                combo = name
                break
        if combo is not None:
            for name in list(t):
                if name != combo:
                    t[name] = set()
        return t

    _bm.get_activation_tables = patched
    _bm._act_tables_patched = True

# test.py can flip these for profiling runs
TRACE = False
TRACE_KWARGS = {}
LAST_RESULTS = None


def _build_program():
    _patch_act_tables()
    dt = mybir.dt
    nc = bacc.Bacc("TRN2", target_bir_lowering=False, debug=False,
                   num_devices=NCORES)

    xt = nc.dram_tensor("xt", [D, ROWS], dt.bfloat16, kind="ExternalInput").ap()
    mt = nc.dram_tensor("mt", [D, NT], dt.bfloat16, kind="ExternalInput").ap()
    vt = nc.dram_tensor("vt", [NT, D], dt.bfloat16, kind="ExternalInput").ap()
    ind = nc.dram_tensor("ind", [128, NTT * H], dt.bfloat16, kind="ExternalInput").ap()
    sel = nc.dram_tensor("sel", [H, NTT * 128], dt.bfloat16, kind="ExternalInput").ap()
    ebias = nc.dram_tensor("ebias", [128, NTT], dt.float32, kind="ExternalInput").ap()
    out = nc.dram_tensor("out", [ROWS, D], dt.bfloat16, kind="ExternalOutput").ap()

    with tile.TileContext(nc) as tc, ExitStack() as ctx:
        wpool = ctx.enter_context(tc.tile_pool(name="weights", bufs=1))
        xpool = ctx.enter_context(tc.tile_pool(name="xt", bufs=2))
        ptpool = ctx.enter_context(tc.tile_pool(name="pt", bufs=2))
        pnpool = ctx.enter_context(tc.tile_pool(name="pn", bufs=2))
        rcpool = ctx.enter_context(tc.tile_pool(name="recip", bufs=2))
        otpool = ctx.enter_context(tc.tile_pool(name="osb", bufs=3))
        # 8 PSUM banks: scores(3) + sums(1) + bcast(2) + pvw out(2)
        spsum = ctx.enter_context(tc.tile_pool(name="spsum", bufs=3, space="PSUM"))
        supsum = ctx.enter_context(tc.tile_pool(name="supsum", bufs=1, space="PSUM"))
        rbpsum = ctx.enter_context(tc.tile_pool(name="rbpsum", bufs=2, space="PSUM"))
        opsum = ctx.enter_context(tc.tile_pool(name="opsum", bufs=2, space="PSUM"))

        # --- resident weights, loaded on the ACT hwdge queue so they overlap
        # the first xt load (sync queue).  mt is split per k-chunk so the
        # first score matmuls only wait for chunk 0.
        ebias_t = wpool.tile([128, NTT], dt.float32, tag="ebias")
        nc.scalar.dma_start(out=ebias_t[:], in_=ebias[:, :])
        mt_ts = []
        for kc in range(ND):
            mtc = wpool.tile([128, NT], dt.bfloat16, tag=f"mt{kc}")
            nc.scalar.dma_start(out=mtc[:], in_=mt[kc * 128:(kc + 1) * 128, :])
            mt_ts.append(mtc)
        ind_t = wpool.tile([128, NTT * H], dt.bfloat16, tag="ind")
        nc.scalar.dma_start(out=ind_t[:], in_=ind[:, :])
        sel_t = wpool.tile([H, NTT * 128], dt.bfloat16, tag="sel")
        nc.scalar.dma_start(out=sel_t[:], in_=sel[:, :])
        vt_t = wpool.tile([128, NTT * D], dt.bfloat16, tag="vt")
        for st in range(NTT):
            nc.scalar.dma_start(out=vt_t[:, st * D:(st + 1) * D],
                                in_=vt[st * 128:(st + 1) * 128, :])

        def pvw_quarter(pat, prt, rc):
            """One 128-row chunk of the previous rowtile's P_norm @ Vt."""
            ot = otpool.tile([128, D], dt.bfloat16, tag="ot")
            for oc in range(2):
                op_ = opsum.tile([128, RT], dt.float32, tag="op")
                for st in range(NTT):
                    nc.tensor.matmul(
                        op_[:],
                        lhsT=pat[:, st * RT + rc * 128: st * RT + (rc + 1) * 128],
                        rhs=vt_t[:, st * D + oc * RT: st * D + (oc + 1) * RT],
                        start=(st == 0), stop=(st == NTT - 1),
                    )
                if oc == 0:
                    nc.scalar.copy(ot[:, oc * RT:(oc + 1) * RT], op_[:])
                else:
                    nc.vector.tensor_copy(ot[:, oc * RT:(oc + 1) * RT], op_[:])
            dma_eng = nc.sync if rc % 2 == 0 else nc.scalar
            dma_eng.dma_start(
                out=out[prt * RT + rc * 128: prt * RT + (rc + 1) * 128, :],
                in_=ot[:],
            )

        def load_xt(rt):
            ts = []
            for kc in range(ND):
                t = xpool.tile([128, RT], dt.bfloat16, tag=f"xt{kc}")
                nc.sync.dma_start(
                    out=t[:],
                    in_=xt[kc * 128:(kc + 1) * 128, rt * RT:(rt + 1) * RT])
                ts.append(t)
            return ts

        def score_group(xts, pt_t, st):
            sp = spsum.tile([128, RT], dt.float32, tag="sp")
            for kc in range(ND):
                nc.tensor.matmul(
                    sp[:],
                    lhsT=mt_ts[kc][:, st * 128:(st + 1) * 128],
                    rhs=xts[kc][:],
                    start=(kc == 0), stop=(kc == ND - 1),
                )
            nc.scalar.activation(pt_t[:, st * RT:(st + 1) * RT], sp[:],
                                 mybir.ActivationFunctionType.Exp,
                                 bias=ebias_t[:, st:st + 1])

        NPULL = 4  # next-rowtile score groups pulled in to cover 1/s latency
        xts = load_xt(0)
        pt_t = ptpool.tile([128, NTT * RT], dt.bfloat16, tag="pt")
        nxt = (xts, pt_t)
        prev = None
        for rt in range(NRT):
            xts, pt_t = nxt
            for st in range(0 if rt == 0 else NPULL, NTT):
                score_group(xts, pt_t, st)

            # --- all-head row sums -> [16, 512] PSUM
            su = supsum.tile([H, RT], dt.float32, tag="su")
            for st in range(NTT):
                nc.tensor.matmul(
                    su[:],
                    lhsT=ind_t[:, st * H:(st + 1) * H],
                    rhs=pt_t[:, st * RT:(st + 1) * RT],
                    start=(st == 0), stop=(st == NTT - 1),
                )
            # 1/s = exp(-ln s) on ACT (writes bf16 directly); the DVE
            # InstReciprocal measures ~3.3us and sits on the critical path.
            rc_f = rcpool.tile([H, RT], dt.float32, tag="rcf")
            nc.scalar.activation(rc_f[:], su[:],
                                 mybir.ActivationFunctionType.Ln)
            rc_b = rcpool.tile([H, RT], dt.bfloat16, tag="rcb")
            nc.scalar.activation(rc_b[:], rc_f[:],
                                 mybir.ActivationFunctionType.Exp,
                                 scale=-1.0)

            # cover the 1/s latency with independent PE work: the next
            # rowtile's first score groups, then PVW of the previous rowtile
            if rt + 1 < NRT:
                nxts = load_xt(rt + 1)
                npt = ptpool.tile([128, NTT * RT], dt.bfloat16, tag="pt")
                for st in range(NPULL):
                    score_group(nxts, npt, st)
                nxt = (nxts, npt)
            if prev is not None:
                pvw_quarter(prev[1], prev[0], 0)
                pvw_quarter(prev[1], prev[0], 1)

            # --- broadcast 1/s to token partitions, normalize P
            pn_t = pnpool.tile([128, NTT * RT], dt.bfloat16, tag="pn")
            for st in range(NTT):
                rb = rbpsum.tile([128, RT], dt.float32, tag="rb")
                nc.tensor.matmul(
                    rb[:],
                    lhsT=sel_t[:, st * 128:(st + 1) * 128],
                    rhs=rc_b[:],
                    start=True, stop=True,
                )
                nc.vector.tensor_mul(
                    pn_t[:, st * RT:(st + 1) * RT],
                    pt_t[:, st * RT:(st + 1) * RT], rb[:],
                )

            if prev is not None:
                for rc4 in range(2, 4):
                    pvw_quarter(prev[1], prev[0], rc4)
            prev = (rt, pn_t)

        # drain: PVW of the final rowtile
        for rc4 in range(4):
            pvw_quarter(prev[1], prev[0], rc4)

    nc.compile()
    return nc


def _get_program():
    if "p" not in _PROG_CACHE:
        _PROG_CACHE["p"] = _build_program()
    return _PROG_CACHE["p"]


def _prep_inputs(x, te, mask, Wq, Wk, Wv, Wo, Wst):
    """Host-side fp32 weight folding + per-core shard maps."""
    Weff = ((Wst[:D] + Wst[D:]) @ Wo).astype(np.float32)

    # packed-token-axis structure: global row g -> head g//HS, token g%HS
    g = np.arange(NT)
    head_of = g // HS
    tok_of = g % HS
    real = tok_of < TT

    ind_np = np.zeros((128, NTT * H), np.float32)
    sel_np = np.zeros((H, NTT * 128), np.float32)
    for st in range(NTT):
        for p in range(128):
            gg = st * 128 + p
            if real[gg]:
                ind_np[p, st * H + head_of[gg]] = 1.0
                sel_np[head_of[gg], st * 128 + p] = 1.0

    mt_b, vt_b, eb_b = [], [], []
    for b in range(B):
        K = ((te[b] @ Wk).reshape(TT, G, HD) * SCALE).astype(np.float32)
        V = (te[b] @ Wv).reshape(TT, G, HD).astype(np.float32)
        Mp = np.zeros((D, NT), np.float32)
        Vp = np.zeros((NT, D), np.float32)
        for h in range(H):
            gq = h // HPG
            Mp[:, h * HS:h * HS + TT] = Wq[:, h * HD:(h + 1) * HD] @ K[:, gq, :].T
            Vp[h * HS:h * HS + TT] = V[:, gq, :] @ Weff[h * HD:(h + 1) * HD, :]
        mt_b.append(Mp.astype(BF16))
        vt_b.append(Vp.astype(BF16))
        # exp bias: 0 for attended tokens, -30 for masked/pad rows
        eb = np.full(NT, -30.0, np.float32)
        eb[real] = np.where(mask[b][tok_of[real]], 0.0, -30.0)
        eb_b.append(np.ascontiguousarray(eb.reshape(NTT, 128).T))

    ind_np = ind_np.astype(BF16)
    sel_np = sel_np.astype(BF16)

    in_maps = []
    for c in range(NCORES):
        b = c // (NCORES // B)
        fr = (c % (NCORES // B)) * FPC
        xc = x[b, fr:fr + FPC].reshape(ROWS, D).astype(BF16)
        in_maps.append({
            "xt": np.ascontiguousarray(xc.T),
            "mt": mt_b[b],
            "vt": vt_b[b],
            "ind": ind_np,
            "sel": sel_np,
            "ebias": eb_b[b],
        })
    return in_maps


def kernel(x, text_embeddings, padding_mask, use_mqa=0, use_qk_norm=0,
           Wq=None, Wk=None, Wv=None, Wo=None, Wst=None):
    global LAST_RESULTS
    x = np.asarray(x, np.float32)
    te = np.asarray(text_embeddings, np.float32)
    mask = np.asarray(padding_mask).astype(bool)
    Wq = np.asarray(Wq, np.float32)
    Wk = np.asarray(Wk, np.float32)
    Wv = np.asarray(Wv, np.float32)
    Wo = np.asarray(Wo, np.float32)
    Wst = np.asarray(Wst, np.float32)
    assert x.shape == (B, T, HW, D) and te.shape == (B, TT, D)

    in_maps = _prep_inputs(x, te, mask, Wq, Wk, Wv, Wo, Wst)
    nc = _get_program()

    res = run_bass_kernel_spmd(nc, in_maps, list(range(NCORES)),
                               trace=TRACE, **TRACE_KWARGS)
    LAST_RESULTS = res

    outp = np.empty((B, T, HW, D), np.float32)
    for c in range(NCORES):
        b = c // (NCORES // B)
        fr = (c % (NCORES // B)) * FPC
        outp[b, fr:fr + FPC] = res.results[c]["out"].astype(np.float32).reshape(FPC, HW, D)
    return outp
